# revision 3
# baseline (speedup 1.0000x reference)
"""Trainium2 Bass kernel for nn_HVGuardModel (dense MoE routing).

Reference math (B=65536, D=1024, E=8, H=128, C1=64, NC=2):
    gw  = softmax(x @ Wg + bg)                      [B, E]
    h   = relu(einsum('bd,edh', x, We1) + be1)      [B, E, H]
    eo  = einsum('beh,eho', h, We2) + be2           [B, E, H]
    mix = einsum('be,beh', gw, eo)                  [B, H]
    out = relu(mix @ Wc1 + bc1) @ Wc2 + bc2         [B, NC]

Strategy: pure data-parallel over 8 cores (8192 rows each), feature-major
[feature, batch] layout, zero device transposes.

Algebraic folds (host side), as in the fp32r baseline:
  * V = We2 @ Wc1 per expert ([E*H, 64]) and C = be2 @ Wc1 fold the
    expert-2 + gate-mix + cls-1 chain into one PSUM accumulation.
  * Layer-1 features interleaved f = j*E + e; a replicated-gate weight
    block (Wg columns tiled mod 8) gives per-partition gate scales with
    no cross-partition broadcast.
  * Softmax division deferred to the [64, N] "pre" tile: everything up
    to cls-1 is linear in the *unnormalized* exp(logits), so we multiply
    by 1/s once on [64, N] instead of normalizing [128, N] gate rows.
    This also removes the exp -> denom -> recip chain from the path that
    feeds the layer-1 hs multiplies.

fp8 acceleration (the change vs the fp32r baseline):
  * The two x-contractions (layer-1 grouped GEMM + gate) run as
    fp8e4m3 DoubleRow matmuls: K=256 per instruction at 0.5 cyc/col =
    2x fp32r throughput.
  * e4m3 has 3 mantissa bits (~3.6% rms); single-pass fails the 2e-2
    gate (measured 2.5e-2).  Both operands are split hi+lo:
        x @ W = xh@wh + (xl@wh + xh@wl)      (lo*lo term dropped)
    The cross terms pair naturally into one DoubleRow matmul per
    k-chunk (slots (wl_k, wh_k) x (xh_k, xl_k)).  3 terms = 12 DR
    matmuls per 128-feature block vs 8 fp32r matmuls: 0.75x cycles.
  * W residuals underflow e4m3's subnormal floor (entries ~1/32), so
    weights are prescaled by 4096 before quantization and descaled for
    free via the ACT scale operand on the existing relu/exp.
    Measured end-to-end rel err of this scheme: 9.1e-4 (vs 2.5e-4 for
    the all-fp32r baseline), 20x inside the 2e-2 gate.
  * V / C / denom / cls2 matmuls stay fp32r (1.0 cyc/col is the same
    speed bf16 would be, and exact).
  * Classifier head is software-pipelined one btile behind so its PSUM
    -> DVE -> ACT -> PE chain never stalls the PE stream.

Per 512-column batch tile: 119 matmuls, 33280 PE cycles (vs 42496 for
the fp32r baseline), 11 ACT ops, 11 DVE ops.  PE remains the
bottleneck engine at ~13.9us/tile theoretical (2.4 GHz).
"""

import numpy as np

B = 65536
D = 1024
E = 8
H = 128
C1 = 64
NCLS = 2
NCORES = 8
BLOC = B // NCORES  # 8192
NTILE = 512
KD = D // 128  # 8 k-chunks over D
MH = (E * H) // 128  # 8 feature blocks
SW = 4096.0  # weight prescale for the fp8 hi/lo split

MM_DT = "fp8x3"  # informational; test.py passes it back, run() ignores it

_BUILT = {}


def _build_nc(b_per_core: int):
    import concourse.bacc as bacc
    import concourse.tile as tile
    import concourse.mybir as mybir

    nbt = b_per_core // NTILE
    fp32 = mybir.dt.float32
    fp8 = mybir.dt.float8e4
    f32r = mybir.dt.float32r
    DR = mybir.MatmulPerfMode.DoubleRow
    AF = mybir.ActivationFunctionType
    OP = mybir.AluOpType

    nc = bacc.Bacc("TRN2", target_bir_lowering=False, debug=False)

    xq = nc.dram_tensor("XQ", [128, nbt, 2, KD, NTILE], fp8, kind="ExternalInput")
    wq = nc.dram_tensor("WQ", [128, MH, 2, KD, 128], fp8, kind="ExternalInput")
    wg = nc.dram_tensor("WGQ", [128, 2, KD, 128], fp8, kind="ExternalInput")
    vb = nc.dram_tensor("Vb", [128, MH * C1], f32r, kind="ExternalInput")
    s8 = nc.dram_tensor("S8", [8, C1 + 64], f32r, kind="ExternalInput")
    wc2 = nc.dram_tensor("WC2", [C1, NCLS], f32r, kind="ExternalInput")
    # per-partition bias columns (fp32): 0..7 = be1 block m, 8 = bg_rep,
    # 9 = bc1 (rows 0:64), 10 = bc2 (rows 0:2)
    bcol = nc.dram_tensor("BCOL", [128, 11], fp32, kind="ExternalInput")
    yT = nc.dram_tensor("yT", [NCLS, b_per_core], fp32, kind="ExternalOutput")

    rs = 1.0 / SW

    with tile.TileContext(nc) as tc:
        with (
            tc.tile_pool(name="wpool", bufs=1) as wpool,
            tc.tile_pool(name="xpool", bufs=2) as xpool,
            tc.tile_pool(name="spool", bufs=2) as spool,
            tc.tile_pool(name="hpool", bufs=2) as hpool,
            tc.tile_pool(name="opool", bufs=2) as opool,
            tc.tile_pool(name="ps_gate", bufs=2, space="PSUM") as ps_gate,
            tc.tile_pool(name="ps_srep", bufs=1, space="PSUM") as ps_srep,
            tc.tile_pool(name="ps_h", bufs=2, space="PSUM") as ps_h,
            tc.tile_pool(name="ps_pre", bufs=2, space="PSUM") as ps_pre,
            tc.tile_pool(name="ps_out", bufs=1, space="PSUM") as ps_out,
        ):
            # ---- load weights/constants once, ordered by first use ----
            wgt = wpool.tile([128, 2, KD, 128], fp8, tag="wg")
            bct = wpool.tile([128, 11], fp32, tag="bct")
            s8t = wpool.tile([8, C1 + 64], f32r, tag="s8t")
            wts = [
                wpool.tile([128, 2, KD, 128], fp8, tag=f"w{m}", name=f"w{m}")
                for m in range(MH)
            ]
            vbt = wpool.tile([128, MH * C1], f32r, tag="vbt")
            wc2t = wpool.tile([C1, NCLS], f32r, tag="wc2t")

            def xdma(t):
                xt = xpool.tile([128, 2, KD, NTILE], fp8, tag="x4")
                nc.sync.dma_start(xt[:], xq[:, t])
                return xt

            nc.sync.dma_start(wgt[:], wg[:])
            nc.sync.dma_start(bct[:], bcol[:])
            nc.sync.dma_start(s8t[:], s8[:])
            xk0 = xdma(0)
            for m in range(MH):
                nc.sync.dma_start(wts[m][:], wq[:, m])
            nc.sync.dma_start(vbt[:], vb[:])
            nc.sync.dma_start(wc2t[:], wc2[:])

            c_blk = s8t[:, 0:C1]  # [8, 64]  be2 @ Wc1
            ones64 = s8t[:, C1 : C1 + 64]  # [8, 64] ones -> denom replicate

            def l1_group(wt, pt, xt):
                # 3-term fp8 product of one 128-feature block:
                # main pass wh (x) xh over chunk pairs, then cross pass
                # (wl_k (x) xh_k) + (wh_k (x) xl_k) per chunk.
                for kp in range(KD // 2):
                    nc.tensor.matmul(
                        pt[:], wt[:, 1, 2 * kp : 2 * kp + 2, :],
                        xt[:, 0, 2 * kp : 2 * kp + 2, :],
                        start=(kp == 0), stop=False, perf_mode=DR,
                    )
                for k in range(KD):
                    nc.tensor.matmul(
                        pt[:], wt[:, :, k, :], xt[:, :, k, :],
                        start=False, stop=(k == KD - 1), perf_mode=DR,
                    )

            def cls_head(pp, rinv, t_out):
                # pre2 = pre * (1/s); rp = relu(pre2 + bc1); out = Wc2.T@rp
                pre2 = spool.tile([C1, NTILE], fp32, tag="pre2")
                nc.vector.tensor_tensor(pre2[:], pp[:], rinv[:], op=OP.mult)
                rp = spool.tile([C1, NTILE], f32r, tag="rp")
                nc.scalar.activation(rp[:], pre2[:], AF.Relu, bias=bct[0:C1, 9:10])
                po = ps_out.tile([NCLS, NTILE], fp32, tag="out")
                nc.tensor.matmul(po[:], wc2t[:], rp[:], start=True, stop=True)
                ot = opool.tile([NCLS, NTILE], fp32, tag="o")
                nc.scalar.activation(
                    ot[:], po[:], AF.Identity, bias=bct[0:NCLS, 10:11]
                )
                nc.sync.dma_start(
                    yT[0:NCLS, t_out * NTILE : (t_out + 1) * NTILE], ot[:]
                )

            prev = None  # (ps_pre tile, rinv tile) of previous btile
            for t in range(nbt):
                xt = xk0 if t == 0 else xdma(t)

                # gate logits (PE) -- prev tile's cls head slots in behind
                gp = ps_gate.tile([128, NTILE], fp32, tag="gate")
                l1_group(wgt, gp, xt)

                if prev is not None:
                    cls_head(*prev, t - 1)

                # exp(logit/SW + bg): unnormalized gate weights, all rows
                expg = spool.tile([128, NTILE], f32r, tag="expg")
                nc.scalar.activation(
                    expg[:], gp[:], AF.Exp, bias=bct[:, 8:9], scale=rs
                )

                pp = ps_pre.tile([C1, NTILE], fp32, tag="pre")
                hs = []
                sp = None
                for m in range(MH):
                    hp = ps_h.tile([128, NTILE], fp32, tag="h")
                    l1_group(wts[m], hp, xt)
                    if m == 0:
                        # softmax denominator, replicated over 64 rows;
                        # emitted here so PE reaches it well after exp.
                        sp = ps_srep.tile([C1, NTILE], fp32, tag="srep")
                        nc.tensor.matmul(
                            sp[:], ones64, expg[0:8, :], start=True, stop=True
                        )
                    hr = hpool.tile([128, NTILE], f32r, tag=f"hs{m}")
                    nc.scalar.activation(
                        hr[:], hp[:], AF.Relu, bias=bct[:, m : m + 1], scale=rs
                    )
                    nc.vector.tensor_tensor(hr[:], hr[:], expg[:], op=OP.mult)
                    hs.append(hr)
                    if m == 0:
                        rinv = spool.tile([C1, NTILE], fp32, tag="rinv")
                        nc.vector.reciprocal(rinv[:], sp[:])
                    # V matmuls trail three blocks behind their hs
                    if m >= 3:
                        vm = m - 3
                        nc.tensor.matmul(
                            pp[:], vbt[:, vm * C1 : (vm + 1) * C1], hs[vm][:],
                            start=(vm == 0), stop=False,
                        )
                for vm in range(MH - 3, MH):
                    nc.tensor.matmul(
                        pp[:], vbt[:, vm * C1 : (vm + 1) * C1], hs[vm][:],
                        start=False, stop=False,
                    )
                nc.tensor.matmul(
                    pp[:], c_blk, expg[0:8, :], start=False, stop=True
                )
                prev = (pp, rinv)

            cls_head(*prev, nbt - 1)

    nc.compile()
    return nc


def _get_nc(b_per_core: int):
    if b_per_core not in _BUILT:
        _BUILT[b_per_core] = _build_nc(b_per_core)
    return _BUILT[b_per_core]


def prep_inputs(x, We1, be1, We2, be2, Wg, bg, Wc1, bc1, Wc2, bc2,
                n_cores=NCORES):
    """Host-side packing -> list of per-core input maps."""
    import ml_dtypes

    E4 = ml_dtypes.float8_e4m3  # TRN variant (max normal 240)
    f64 = np.float64
    f32 = np.float32
    b_per_core = x.shape[0] // n_cores
    nbt = b_per_core // NTILE

    # layer-1 weights, feature order f = j*E + e, prescaled hi/lo e4m3
    W1_all = np.transpose(np.asarray(We1, f64), (1, 2, 0)).reshape(D, E * H)
    A = (W1_all * SW).astype(f32)
    wh = A.astype(E4)
    wl = (A - wh.astype(f32)).astype(E4)
    wl_r = wl.reshape(KD, 128, MH, 128).transpose(1, 2, 0, 3)  # [p, m, k, j]
    wh_r = wh.reshape(KD, 128, MH, 128).transpose(1, 2, 0, 3)
    WQ = np.ascontiguousarray(np.stack([wl_r, wh_r], axis=2))  # [128,MH,2,KD,128]

    Wg_rep = np.asarray(Wg, f64)[:, np.arange(128) % E]
    Ag = (Wg_rep * SW).astype(f32)
    wgh = Ag.astype(E4)
    wgl = (Ag - wgh.astype(f32)).astype(E4)
    wgl_r = wgl.reshape(KD, 128, 128).transpose(1, 0, 2)  # [p, k, j]
    wgh_r = wgh.reshape(KD, 128, 128).transpose(1, 0, 2)
    WGQ = np.ascontiguousarray(np.stack([wgl_r, wgh_r], axis=1))  # [128,2,KD,128]

    V = np.einsum(
        "ejk,kc->jec", np.asarray(We2, f64), np.asarray(Wc1, f64)
    ).reshape(E * H, C1)
    Vb = np.ascontiguousarray(
        np.concatenate([V[k * 128 : (k + 1) * 128, :] for k in range(MH)], axis=1)
        .astype(f32)
    )
    Cm = np.asarray(be2, f64) @ np.asarray(Wc1, f64)  # [E, C1]
    S8 = np.ascontiguousarray(
        np.concatenate([Cm, np.ones((E, 64), f64)], axis=1).astype(f32)
    )
    WC2 = np.ascontiguousarray(np.asarray(Wc2, f64).astype(f32))

    bcol = np.zeros((128, 11), np.float32)
    be1_int = np.asarray(be1, f64).T.reshape(E * H)  # f = j*E + e
    for m in range(MH):
        bcol[:, m] = be1_int[m * 128 : (m + 1) * 128]
    bcol[:, 8] = np.asarray(bg, f64)[np.arange(128) % E]
    bcol[0:C1, 9] = np.asarray(bc1, f64)
    bcol[0:NCLS, 10] = np.asarray(bc2, f64)

    xT = np.asarray(x).T.astype(f32)  # [D, B]
    xh = xT.astype(E4)
    xl = (xT - xh.astype(f32)).astype(E4)
    in_maps = []
    for c in range(n_cores):
        sl = slice(c * b_per_core, (c + 1) * b_per_core)
        xh_r = xh[:, sl].reshape(KD, 128, nbt, NTILE).transpose(1, 2, 0, 3)
        xl_r = xl[:, sl].reshape(KD, 128, nbt, NTILE).transpose(1, 2, 0, 3)
        XQ = np.ascontiguousarray(np.stack([xh_r, xl_r], axis=2))
        in_maps.append(
            {
                "XQ": XQ,  # [128, nbt, 2, KD, NTILE] e4m3
                "WQ": WQ,
                "WGQ": WGQ,
                "Vb": Vb,
                "S8": S8,
                "WC2": WC2,
                "BCOL": bcol,
            }
        )
    return in_maps, b_per_core


def run(inputs, mm_dt_name=MM_DT, trace=False):
    """Run on 8 NeuronCores; returns (y [B, 2] fp32, exec_time_ns or None)."""
    from concourse.bass_utils import run_bass_kernel_spmd

    in_maps, b_per_core = prep_inputs(**inputs)
    nc = _get_nc(b_per_core)
    res = run_bass_kernel_spmd(
        nc, in_maps, core_ids=list(range(NCORES)), trace=trace
    )
    y = np.concatenate([r["yT"].T for r in res.results], axis=0)
    return np.ascontiguousarray(y.astype(np.float32)), res.exec_time_ns


def kernel(**inputs):
    y, _ = run(inputs)
    return y
